# revision 7
# baseline (speedup 1.0000x reference)
"""Local-window multi-head attention (window=33) for Trainium2, 8-core SPMD.

Sharding: data-parallel over batch (B=8 -> 1 batch per core). Weights
replicated. Per core: QKV projections (fp32r matmuls), banded local
attention via transposed-score blocks of 96 queries x 128 keys, output
projection, all fused in one Bass/Tile kernel.

Layout notes:
  - x fed transposed (host prep): xT [512, S] so projections contract din
    on partitions.
  - q^T, k^T kept [dout, pos] (head h = rows 64*(h%2) of chunk h//2).
  - scores computed transposed: S^T[kpos, q] = k_h^T.T @ q_h^T, so the
    key-side mask/range penalty is a per-partition bias of the exp on
    ScalarE, and MM2 (P^T as lhsT) gives attn natural [q, d] with the
    softmax denominator from a ones-column appended to v.
  - v projected per 96-query block directly into [kpos 128, head, 65]
    tiles (col 64 = ones), so MM2 needs no K-splits.
"""
import sys
sys.path.insert(0, "/opt/trn_rl_repo")
import numpy as np

B, S, D, H, HD = 8, 4096, 512, 8, 64
WIN, HALF = 33, 16
QB = 96
NB = (S + QB - 1) // QB          # 43 blocks (42 full + 64)
CPB = 6                          # blocks per chunk
NEG = -1e9

_CHUNKS = [list(range(c * CPB, min(NB, (c + 1) * CPB))) for c in range((NB + CPB - 1) // CPB)]

_NC = None


def _qw(j):
    return min(QB, S - QB * j)


def _build():
    import concourse.bacc as bacc
    import concourse.mybir as mybir
    from concourse.tile import TileContext

    F32 = mybir.dt.float32
    F32R = mybir.dt.float32r
    EXP = mybir.ActivationFunctionType.Exp
    MULT = mybir.AluOpType.mult

    nc = bacc.Bacc(None, target_bir_lowering=False)

    xqT = nc.dram_tensor("xqT", [D, S], F32R, kind="ExternalInput")
    xkT = nc.dram_tensor("xkT", [D, S], F32R, kind="ExternalInput")
    xvT = nc.dram_tensor("xvT", [D, S], F32R, kind="ExternalInput")
    wqT = nc.dram_tensor("wqT", [D, D], F32R, kind="ExternalInput")
    wkT = nc.dram_tensor("wkT", [D, D], F32R, kind="ExternalInput")
    wvT = nc.dram_tensor("wvT", [D, D], F32R, kind="ExternalInput")
    woT = nc.dram_tensor("woT", [D, D], F32R, kind="ExternalInput")
    bqc_d = nc.dram_tensor("bqc", [128, 4], F32, kind="ExternalInput")
    bkc_d = nc.dram_tensor("bkc", [128, 4], F32, kind="ExternalInput")
    boe_d = nc.dram_tensor("boeff", [1, D], F32R, kind="ExternalInput")
    pen_d = nc.dram_tensor("pen", [128, NB], F32, kind="ExternalInput")
    band_d = nc.dram_tensor("band8", [128, H * QB], F32R, kind="ExternalInput")
    id_d = nc.dram_tensor("ident", [128, 128], F32R, kind="ExternalInput")
    one_d = nc.dram_tensor("ones", [1, QB], F32R, kind="ExternalInput")
    zpad_d = nc.dram_tensor("zpad", [128, 4, 64], F32R, kind="ExternalInput")
    vone_d = nc.dram_tensor("vones", [128, H, 2], F32R, kind="ExternalInput")
    qzero_d = nc.dram_tensor("qzero", [64, 4, 576], F32R, kind="ExternalInput")
    out_d = nc.dram_tensor("out", [S, D], F32, kind="ExternalOutput")

    def r4(t):  # [512, N] dram -> [128, 4, N] view
        return t[:, :].rearrange("(c p) n -> p c n", p=128)

    with TileContext(nc) as tc:
        with tc.tile_pool(name="const", bufs=1) as cp, \
             tc.tile_pool(name="stage", bufs=2) as stp, \
             tc.tile_pool(name="qk", bufs=2) as qkp, \
             tc.tile_pool(name="vtiles", bufs=8) as vp, \
             tc.tile_pool(name="pt", bufs=3) as ptp, \
             tc.tile_pool(name="small", bufs=3) as smp, \
             tc.tile_pool(name="outp", bufs=3) as outp, \
             tc.tile_pool(name="proj_ps", bufs=2, space="PSUM") as proj_ps, \
             tc.tile_pool(name="st_ps", bufs=2, space="PSUM") as st_ps, \
             tc.tile_pool(name="mm2_ps", bufs=2, space="PSUM") as mm2_ps, \
             tc.tile_pool(name="tr_ps", bufs=1, space="PSUM") as tr_ps, \
             tc.tile_pool(name="op_ps", bufs=1, space="PSUM") as op_ps:

            # ---- constants ----
            wq_sb = cp.tile([128, 4, D], F32R, name="wq_sb")
            wk_sb = cp.tile([128, 4, D], F32R, name="wk_sb")
            wv_sb = cp.tile([128, 4, D], F32R, name="wv_sb")
            wo_sb = cp.tile([128, 4, D], F32R, name="wo_sb")
            nc.sync.dma_start(wq_sb[:], r4(wqT))
            nc.sync.dma_start(wk_sb[:], r4(wkT))
            nc.sync.dma_start(wv_sb[:], r4(wvT))
            nc.sync.dma_start(wo_sb[:], r4(woT))
            bqc = cp.tile([128, 4], F32, name="bqc_sb")
            bkc = cp.tile([128, 4], F32, name="bkc_sb")
            boe = cp.tile([1, D], F32R, name="boe_sb")
            pen = cp.tile([128, NB], F32, name="pen_sb")
            band = cp.tile([128, H, QB], F32R, name="band_sb")
            iden = cp.tile([128, 128], F32R, name="id_sb")
            ones = cp.tile([1, QB], F32R, name="ones_sb")
            nc.sync.dma_start(bqc[:], bqc_d[:, :])
            nc.sync.dma_start(bkc[:], bkc_d[:, :])
            nc.sync.dma_start(boe[:], boe_d[:, :])
            nc.sync.dma_start(pen[:], pen_d[:, :])
            nc.sync.dma_start(band[:], band_d[:, :].rearrange("p (h q) -> p h q", q=QB))
            nc.sync.dma_start(iden[:], id_d[:, :])
            nc.sync.dma_start(ones[:], one_d[:, :])

            # persistent double-buffered qZ: per-head q^T with the other
            # co-projected head's partition half zeroed (lets MM1 run as a
            # full-K=128 matmul at partition base 0 -- fp32r matmuls with
            # base-64 operands fault at runtime)
            qZ_bufs = []
            for bi in range(2):
                qz = cp.tile([128, H, 576], F32R, name=f"qZ{bi}")
                nc.sync.dma_start(qz[0:64, 1:H:2, :], qzero_d[:, :, :])
                nc.sync.dma_start(qz[64:128, 0:H:2, :], qzero_d[:, :, :])
                qZ_bufs.append(qz)

            for ci, blocks in enumerate(_CHUNKS):
                qZ = qZ_bufs[ci % 2]
                j0, j1 = blocks[0], blocks[-1]
                q_lo = QB * j0
                q_hi = min(S, QB * (j1 + 1))
                qwid = q_hi - q_lo                       # 576 or 64
                win_lo = QB * j0 - HALF                  # may be < 0
                win_hi = QB * j1 + 112                   # may be > S
                kwid = win_hi - win_lo                   # 608 or 128
                dlo, dhi = max(0, win_lo), min(S, win_hi)

                # ---- stage x^T slices ----
                xq_st = stp.tile([128, 4, 576], F32R, tag="xq_st", name="xq_st")
                xk_st = stp.tile([128, 4, 608], F32R, tag="xk_st", name="xk_st")
                xv_st = stp.tile([128, 4, 608], F32R, tag="xv_st", name="xv_st")
                nc.sync.dma_start(xq_st[:, :, :qwid], r4(xqT)[:, :, q_lo:q_hi])
                nc.sync.dma_start(xk_st[:, :, dlo - win_lo:dhi - win_lo],
                                  r4(xkT)[:, :, dlo:dhi])
                nc.sync.dma_start(xv_st[:, :, dlo - win_lo:dhi - win_lo],
                                  r4(xvT)[:, :, dlo:dhi])
                if dlo > win_lo:
                    w = dlo - win_lo
                    nc.sync.dma_start(xk_st[:, :, 0:w], zpad_d[:, :, 0:w])
                    nc.sync.dma_start(xv_st[:, :, 0:w], zpad_d[:, :, 0:w])
                if dhi < win_hi:
                    w = win_hi - dhi
                    nc.sync.dma_start(xk_st[:, :, dhi - win_lo:kwid],
                                      zpad_d[:, :, 0:w])
                    nc.sync.dma_start(xv_st[:, :, dhi - win_lo:kwid],
                                      zpad_d[:, :, 0:w])

                # ---- q^T / k^T projections ----
                kT = qkp.tile([128, 4, 608], F32R, tag="kT", name="kT")
                for dst, src, w_sb, b_sb, wid in (
                        (qZ, xq_st, wq_sb, bqc, qwid), (kT, xk_st, wk_sb, bkc, kwid)):
                    ntile = 2 if wid > 512 else 1
                    nw = wid // ntile
                    for dc in range(4):
                        for t in range(ntile):
                            ps = proj_ps.tile([128, 512], F32, tag="proj", name="pps")
                            for k in range(4):
                                nc.tensor.matmul(ps[:, :nw],
                                                 w_sb[:, k, 128 * dc:128 * dc + 128],
                                                 src[:, k, nw * t:nw * t + nw],
                                                 start=(k == 0), stop=(k == 3))
                            sl = slice(nw * t, nw * t + nw)
                            if dst is qZ:
                                nc.vector.tensor_scalar_add(
                                    qZ[0:64, 2 * dc, sl], ps[0:64, :nw],
                                    b_sb[0:64, dc:dc + 1])
                                nc.vector.tensor_scalar_add(
                                    qZ[64:128, 2 * dc + 1, sl], ps[64:128, :nw],
                                    b_sb[64:128, dc:dc + 1])
                            else:
                                nc.scalar.add(
                                    dst[:, dc, sl], ps[:, :nw],
                                    b_sb[:, dc:dc + 1])

                # ---- v projection: per block, 96-strided [128, H, 65] ----
                vts = {}
                for j in blocks:
                    vloc = (QB * j - HALF) - win_lo
                    ps = proj_ps.tile([128, 512], F32, tag="proj", name="vps")
                    for k in range(4):
                        nc.tensor.matmul(ps[:],
                                         xv_st[:, k, vloc:vloc + 128],
                                         wv_sb[:, k, :],
                                         start=(k == 0), stop=(k == 3))
                    vt = vp.tile([128, H, 66], F32R, tag="vt", name="vt")
                    nc.vector.tensor_copy(
                        vt[:, :, 0:64], ps[:].rearrange("p (h d) -> p h d", d=64))
                    nc.sync.dma_start(vt[:, :, 64:66], vone_d[:, :, :])
                    vts[j] = vt

                # ---- attention blocks ----
                for j in blocks:
                    qw = _qw(j)
                    qloc = QB * j - q_lo
                    kloc = (QB * j - HALF) - win_lo
                    pT = ptp.tile([128, H, QB], F32R, tag="pT", name="pT")
                    gps = []
                    for g in range(2):
                        st = st_ps.tile([128, 4, QB], F32, tag="st", name="st")
                        for hi in range(4):
                            h = 4 * g + hi
                            nc.tensor.matmul(
                                st[:, hi, :qw],
                                kT[:, h // 2, kloc:kloc + 128],
                                qZ[:, h, qloc:qloc + qw],
                                start=True, stop=True)
                        nc.scalar.activation(pT[:, 4 * g:4 * g + 4, :qw],
                                             st[:, :, :qw], EXP,
                                             bias=pen[:, j:j + 1], scale=1.0)
                    nc.vector.tensor_tensor(out=pT[:, :, :qw], in0=pT[:, :, :qw],
                                            in1=band[:, :, :qw], op=MULT)
                    att = smp.tile([QB, D], F32R, tag="att", name="att")
                    attv = att.rearrange("q (h d) -> q h d", d=64)
                    rc = smp.tile([QB, H], F32, tag="rc", name="rc")
                    for g in range(2):
                        m2 = mm2_ps.tile([QB, 4, 66], F32, tag="m2", name="m2")
                        for hi in range(4):
                            h = 4 * g + hi
                            nc.tensor.matmul(m2[:qw, hi, :], pT[:, h, :qw],
                                             vts[j][:, h, :], start=True, stop=True)
                        gps.append(m2)
                    for g in range(2):
                        nc.vector.reciprocal(rc[:qw, 4 * g:4 * g + 4],
                                             gps[g][:qw, :, 64])
                    for g in range(2):
                        nc.vector.tensor_tensor(
                            out=attv[:qw, 4 * g:4 * g + 4, :],
                            in0=gps[g][:qw, :, 0:64],
                            in1=rc[:qw, 4 * g:4 * g + 4].unsqueeze(2).to_broadcast(
                                (qw, 4, 64)),
                            op=MULT)
                    # transpose attn -> [dcat, q]
                    tr = tr_ps.tile([128, 4, QB], F32R, tag="tr", name="tr")
                    for i in range(4):
                        nc.tensor.transpose(tr[:, i, :qw],
                                            att[:qw, 128 * i:128 * i + 128],
                                            iden[:qw, :qw])
                    atT = smp.tile([128, 4, QB], F32R, tag="atT", name="atT")
                    nc.scalar.copy(atT[:, :, :qw], tr[:, :, :qw])
                    # output projection + bias
                    op = op_ps.tile([QB, D], F32, tag="op", name="op")
                    for i in range(4):
                        nc.tensor.matmul(op[:qw, :], atT[:, i, :qw], wo_sb[:, i, :],
                                         start=(i == 0), stop=False)
                    nc.tensor.matmul(op[:qw, :], ones[:, :qw], boe[:, :],
                                     start=False, stop=True)
                    osb = outp.tile([QB, D], F32, tag="osb", name="osb")
                    nc.scalar.copy(osb[:qw, :], op[:qw, :])
                    nc.sync.dma_start(out_d[QB * j:QB * j + qw, :], osb[:qw, :])

    nc.finalize()
    return nc


def _host_consts():
    rr = np.arange(128)[:, None]
    qq = np.arange(QB)[None, :]
    band = (((rr - qq) >= 0) & ((rr - qq) <= 32)).astype(np.float32)
    band8 = np.tile(band, (1, H))
    ident = np.eye(128, dtype=np.float32)
    onesr = np.ones((1, QB), np.float32)
    return band8, ident, onesr


def _get_nc():
    global _NC
    if _NC is None:
        _NC = _build()
    return _NC


def _prep_inmaps(query, key, value, mask, Wq, bq, Wk, bk, Wv, bv, Wo, bo):
    query = np.asarray(query, np.float32)
    key = np.asarray(key, np.float32)
    value = np.asarray(value, np.float32)
    mask = np.asarray(mask)
    Wq, bq = np.asarray(Wq, np.float32), np.asarray(bq, np.float32)
    Wk, bk = np.asarray(Wk, np.float32), np.asarray(bk, np.float32)
    Wv, bv = np.asarray(Wv, np.float32), np.asarray(bv, np.float32)
    Wo, bo = np.asarray(Wo, np.float32), np.asarray(bo, np.float32)

    band8, ident, onesr = _host_consts()
    boeff = (Wo @ bv + bo).reshape(1, D).astype(np.float32)
    jj = np.arange(NB)[None, :]
    rr = np.arange(128)[:, None]
    pos = QB * jj - HALF + rr                      # [128, NB]
    valid = (pos >= 0) & (pos < S)
    posc = np.clip(pos, 0, S - 1)

    common = {
        "wqT": np.ascontiguousarray(Wq.T), "wkT": np.ascontiguousarray(Wk.T),
        "wvT": np.ascontiguousarray(Wv.T), "woT": np.ascontiguousarray(Wo.T),
        "bqc": np.ascontiguousarray(bq.reshape(4, 128).T),
        "bkc": np.ascontiguousarray(bk.reshape(4, 128).T),
        "boeff": boeff, "band8": band8, "ident": ident, "ones": onesr,
        "zpad": np.zeros((128, 4, 64), np.float32),
        "vones": np.ones((128, H, 2), np.float32),
        "qzero": np.zeros((64, 4, 576), np.float32),
    }
    in_maps = []
    for b in range(B):
        pen = np.where(valid & ~mask[b][posc], 0.0, NEG).astype(np.float32)
        in_maps.append(dict(
            common,
            xqT=np.ascontiguousarray(query[b].T),
            xkT=np.ascontiguousarray(key[b].T),
            xvT=np.ascontiguousarray(value[b].T),
            pen=pen,
        ))
    return in_maps


def kernel(**inputs):
    from concourse.bass_utils import run_bass_kernel_spmd
    in_maps = _prep_inmaps(**inputs)
    res = run_bass_kernel_spmd(_get_nc(), in_maps, core_ids=list(range(8)))
    return np.stack([res.results[c]["out"] for c in range(B)], axis=0)
